# revision 52
# baseline (speedup 1.0000x reference)
"""AttentionCritic forward kernel for 8 Trainium2 NeuronCores.

Math (live part of the reference; the bi-GRU "hard attention" branch and the
1-element softmax are dead code):
    x     = relu(obs @ enc_w.T + enc_b)
    h_out = GRUCell(x, h)
    v     = relu(h_out @ v_w.T + v_b)
    out   = [h_out, v] @ dec_w.T + dec_b
    return out, h_out

Strategy: batch (65535 rows, padded to 65536) is split over 8 cores (8192
rows each). All compute is done feature-major (features on SBUF partitions,
batch streaming as the matmul free dim), so no transposes are needed on the
device: the host hands the kernel transposed bf16 inputs and re-transposes
the fp32 outputs.
"""

import numpy as np
import ml_dtypes

BF16 = ml_dtypes.bfloat16

N_CORES = 8
B_FULL = 65535
B_PAD = 65536
B_CORE = B_PAD // N_CORES  # 8192
COLS = 1024                # batch columns processed per loop iteration
N_ITERS = B_CORE // COLS

IN_DIM = 256
RNN = 128
ATT = 64
N_ACT = 10

# packed bf16 weight tensor column layout (all [K, M] matmul lhsT blocks)
W_EW0 = 0      # enc_w.T rows 0:128    [128, 128]
W_EW1 = 128    # enc_w.T rows 128:256  [128, 128]
W_XR = 256     # gru_w_ih.T cols 0:128
W_XZ = 384
W_XN = 512
W_HR = 640     # gru_w_hh.T cols 0:128
W_HZ = 768
W_HN = 896
W_TOT = 1024

# packed fp32 weight tensor (v/dec matmuls run fp32 for output precision).
# v_w.T and dec_w.T[0:128] share the rhs (h_out^T), so they are fused into one
# [128, 74] stationary block: out rows 0:64 = v preact, rows 64:74 = dec part.
F_VO = 0       # [v_w.T | dec_w.T rows 0:128] -> [128, 74]
F_DL = 74      # dec_w.T rows 128:192 -> [64, 10] (on partitions 0:64)
F_TOT = 84

# bias tensor [128, 8] fp32 column layout
B_ENC = 0      # enc_b
B_R = 1        # gru_b_ih[0:128] + gru_b_hh[0:128]
B_Z = 2
B_IHN = 3      # gru_b_ih[256:384]
B_HHN = 4      # gru_b_hh[256:384]
B_V = 5        # v_b on rows 0:64
B_DEC = 6      # dec_b on rows 0:10

_cached = {}


def _build_bass():
    if "nc" in _cached:
        return _cached["nc"]
    import concourse.bass as bass
    import concourse.bacc as bacc
    import concourse.tile as tile
    from concourse import mybir

    f32 = mybir.dt.float32
    bf16 = mybir.dt.bfloat16
    ALU = mybir.AluOpType
    ACTF = mybir.ActivationFunctionType

    nc = bacc.Bacc("TRN2", target_bir_lowering=False, debug=False)
    obsT = nc.declare_dram_parameter("obsT", [IN_DIM, B_CORE], bf16, isOutput=False)
    hT = nc.declare_dram_parameter("hT", [RNN, B_CORE], bf16, isOutput=False)
    wts_d = nc.declare_dram_parameter("wts", [128, W_TOT], bf16, isOutput=False)
    wtf_d = nc.declare_dram_parameter("wtf", [128, F_TOT], bf16, isOutput=False)
    bias_d = nc.declare_dram_parameter("bias", [128, 8], f32, isOutput=False)
    h_outT = nc.declare_dram_parameter("h_outT", [RNN, B_CORE], f32, isOutput=True)
    outT = nc.declare_dram_parameter("outT", [N_ACT, B_CORE], f32, isOutput=True)

    LOAD_COLS = 2048  # input DMA granularity (2 compute iterations)

    with tile.TileContext(nc) as tc:
        with (
            tc.tile_pool(name="const", bufs=1) as const,
            tc.tile_pool(name="io", bufs=2) as io,
            tc.tile_pool(name="work", bufs=3) as work,
            tc.tile_pool(name="psum", bufs=4, space="PSUM") as psum,
        ):
            wt = const.tile([128, W_TOT], bf16)
            wf = const.tile([128, F_TOT], bf16)
            bt = const.tile([128, 8], f32)
            nc.sync.dma_start(out=wt, in_=wts_d[:, :])
            nc.sync.dma_start(out=wf, in_=wtf_d[:, :])
            nc.sync.dma_start(out=bt, in_=bias_d[:, :])

            halves = [slice(0, 512), slice(512, 1024)]

            # PE warm-up: dummy matmuls on the weight tile while the first
            # input DMAs are in flight. Keeps the HAM clock gate at 8/8 so the
            # first real iterations run at 2.4 GHz instead of 1.2.
            ps_w = psum.tile([128, 512], f32, tag="ps", name="ps_w")
            for _ in range(12):
                nc.tensor.matmul(ps_w, wt[:, 0:128], wt[:, 0:512],
                                 start=True, stop=True)
            warm_sink = work.tile([128, 512], f32, name="warm_sink")
            nc.scalar.activation(warm_sink, ps_w, ACTF.Relu)

            def stage_b_mm(hout):
                # previous iteration's fused v+dec matmul; emitted while this
                # iteration's xt is in flight so the PE never waits on hout.
                # fused: rows 0:64 = v preact, rows 64:74 = dec_w[:, :128] part
                ps_vo = psum.tile([74, COLS], f32, tag="ps", name="ps_vo")
                for s in halves:
                    nc.tensor.matmul(ps_vo[:, s], wf[:, F_VO:F_VO + 74],
                                     hout[:, s], start=True, stop=True)
                vt = work.tile([64, COLS], bf16, name="vt")
                nc.scalar.activation(vt, ps_vo[0:64, :], ACTF.Relu,
                                     bias=bt[0:64, B_V:B_V + 1])
                return ps_vo, vt

            def stage_b_fin(ps_vo, vt, csl):
                # accumulates onto the closed group above: has_written persists
                # on HW until the next start=True; skip the sim-only check
                for s in halves:
                    nc.tensor.matmul(ps_vo[64:74, s], wf[0:64, F_DL:F_DL + 10],
                                     vt[:, s], start=False, stop=True,
                                     skip_group_check=True)
                ot = work.tile([10, COLS], f32, name="ot")
                nc.vector.tensor_scalar_add(ot, ps_vo[64:74, :],
                                            bt[0:10, B_DEC:B_DEC + 1])
                nc.sync.dma_start(out=outT[:, csl], in_=ot)

            obs0 = obs1 = htl = None
            pending = None
            pending2 = None
            for it in range(N_ITERS):
                c0 = it * COLS
                csl = slice(c0, c0 + COLS)

                if it % (LOAD_COLS // COLS) == 0:
                    lsl = slice(c0, c0 + LOAD_COLS)
                    obs0 = io.tile([128, LOAD_COLS], bf16, tag="obs0")
                    obs1 = io.tile([128, LOAD_COLS], bf16, tag="obs1")
                    htl = io.tile([128, LOAD_COLS], bf16, tag="htl")
                    nc.sync.dma_start(out=obs0, in_=obsT[0:128, lsl])
                    nc.sync.dma_start(out=obs1, in_=obsT[128:256, lsl])
                    nc.sync.dma_start(out=htl, in_=hT[:, lsl])
                w0 = (it % (LOAD_COLS // COLS)) * COLS
                o0 = obs0[:, w0:w0 + COLS]
                o1 = obs1[:, w0:w0 + COLS]
                ht = htl[:, w0:w0 + COLS]

                # x^T = relu(enc_w @ obs^T + enc_b), bf16
                # (matmuls emitted weight-major so LDWEIGHTS can be reused)
                ps_x = psum.tile([128, COLS], f32, tag="ps", name="ps_x")
                for s in halves:
                    nc.tensor.matmul(ps_x[:, s], wt[:, W_EW0:W_EW0 + 128],
                                     o0[:, s], start=True, stop=False)
                for s in halves:
                    nc.tensor.matmul(ps_x[:, s], wt[:, W_EW1:W_EW1 + 128],
                                     o1[:, s], start=False, stop=True)
                xt = work.tile([128, COLS], bf16, name="xt")
                nc.vector.tensor_scalar(out=xt, in0=ps_x,
                                        scalar1=bt[:, B_ENC:B_ENC + 1], scalar2=0.0,
                                        op0=ALU.add, op1=ALU.max)

                # GRU r/z pre-activations: h-side first (doesn't need xt)
                ps_r = psum.tile([128, COLS], f32, tag="ps", name="ps_r")
                ps_z = psum.tile([128, COLS], f32, tag="ps", name="ps_z")
                for s in halves:
                    nc.tensor.matmul(ps_r[:, s], wt[:, W_HR:W_HR + 128], ht[:, s],
                                     start=True, stop=False)
                for s in halves:
                    nc.tensor.matmul(ps_z[:, s], wt[:, W_HZ:W_HZ + 128], ht[:, s],
                                     start=True, stop=False)
                for s in halves:
                    nc.tensor.matmul(ps_r[:, s], wt[:, W_XR:W_XR + 128], xt[:, s],
                                     start=False, stop=True)
                for s in halves:
                    nc.tensor.matmul(ps_z[:, s], wt[:, W_XZ:W_XZ + 128], xt[:, s],
                                     start=False, stop=True)
                r = work.tile([128, COLS], bf16, name="r")
                nc.scalar.activation(r, ps_r, ACTF.Sigmoid, bias=bt[:, B_R:B_R + 1])
                z = work.tile([128, COLS], f32, name="z")
                nc.scalar.activation(z, ps_z, ACTF.Sigmoid, bias=bt[:, B_Z:B_Z + 1])
                # off-critical-path pieces of h_out = (1-z)*n + z*h
                z1 = work.tile([128, COLS], bf16, name="z1")
                nc.gpsimd.tensor_scalar(out=z1, in0=z, scalar1=-1.0, scalar2=1.0,
                                        op0=ALU.mult, op1=ALU.add)
                zh = work.tile([128, COLS], f32, name="zh")
                nc.gpsimd.tensor_mul(zh, z, ht)

                ps_in = psum.tile([128, COLS], f32, tag="ps", name="ps_in")
                ps_hn = psum.tile([128, COLS], f32, tag="ps", name="ps_hn")
                for s in halves:
                    nc.tensor.matmul(ps_hn[:, s], wt[:, W_HN:W_HN + 128], ht[:, s],
                                     start=True, stop=True)
                for s in halves:
                    nc.tensor.matmul(ps_in[:, s], wt[:, W_XN:W_XN + 128], xt[:, s],
                                     start=True, stop=True)

                # decoder deferred TWO iterations: hob(i-2) is guaranteed
                # complete, so the PE never waits on the DVE tail chain
                if pending2 is not None:
                    bmm = stage_b_mm(pending2[0])
                    stage_b_fin(*bmm, pending2[1])

                # n = tanh(i_n + b_ihn + r * (h_n + b_hhn))
                t = work.tile([128, COLS], f32, name="t")
                nc.vector.scalar_tensor_tensor(out=t, in0=ps_hn,
                                               scalar=bt[:, B_HHN:B_HHN + 1], in1=r,
                                               op0=ALU.add, op1=ALU.mult)
                t2 = work.tile([128, COLS], f32, name="t2")
                nc.vector.tensor_add(t2, t, ps_in)
                n = work.tile([128, COLS], bf16, name="n")
                nc.scalar.activation(n, t2, ACTF.Tanh, bias=bt[:, B_IHN:B_IHN + 1])

                # h_out = (1-z)*n + z*h
                zn = work.tile([128, COLS], f32, name="zn")
                nc.vector.tensor_mul(zn, z1, n)
                hout = work.tile([128, COLS], f32, name="hout")
                nc.gpsimd.tensor_add(hout, zn, zh)
                nc.sync.dma_start(out=h_outT[:, csl], in_=hout)
                # bf16 twin for the decoder matmul rhs; ACT cast is cheap
                # and hob is only consumed two iterations later
                hob = work.tile([128, COLS], bf16, name="hob")
                nc.scalar.activation(hob, hout, ACTF.Copy)

                pending2 = pending
                pending = (hob, csl)

            for p in (pending2, pending):
                bmm = stage_b_mm(p[0])
                stage_b_fin(*bmm, p[1])

    nc.compile()  # bacc passes: split multi-waits into event semaphores etc.
    _cached["nc"] = nc
    return nc


def _pack_params(inputs):
    f32 = np.float32
    wts = np.zeros((128, W_TOT), f32)
    enc_wT = inputs["enc_w"].T.astype(f32)            # [256, 128]
    wts[:, W_EW0:W_EW0 + 128] = enc_wT[0:128]
    wts[:, W_EW1:W_EW1 + 128] = enc_wT[128:256]
    w_ihT = inputs["gru_w_ih"].T.astype(f32)          # [128, 384]
    w_hhT = inputs["gru_w_hh"].T.astype(f32)
    wts[:, W_XR:W_XR + 128] = w_ihT[:, 0:128]
    wts[:, W_XZ:W_XZ + 128] = w_ihT[:, 128:256]
    wts[:, W_XN:W_XN + 128] = w_ihT[:, 256:384]
    wts[:, W_HR:W_HR + 128] = w_hhT[:, 0:128]
    wts[:, W_HZ:W_HZ + 128] = w_hhT[:, 128:256]
    wts[:, W_HN:W_HN + 128] = w_hhT[:, 256:384]

    wtf = np.zeros((128, F_TOT), f32)
    wtf[:, F_VO:F_VO + 64] = inputs["v_w"].T.astype(f32)
    dec_wT = inputs["dec_w"].T.astype(f32)            # [192, 10]
    wtf[:, F_VO + 64:F_VO + 74] = dec_wT[0:128]
    wtf[0:64, F_DL:F_DL + 10] = dec_wT[128:192]
    wtf = wtf.astype(BF16)

    bias = np.zeros((128, 8), f32)
    bias[:, B_ENC] = inputs["enc_b"]
    b_ih = inputs["gru_b_ih"].astype(f32)
    b_hh = inputs["gru_b_hh"].astype(f32)
    bias[:, B_R] = b_ih[0:128] + b_hh[0:128]
    bias[:, B_Z] = b_ih[128:256] + b_hh[128:256]
    bias[:, B_IHN] = b_ih[256:384]
    bias[:, B_HHN] = b_hh[256:384]
    bias[0:64, B_V] = inputs["v_b"]
    bias[0:10, B_DEC] = inputs["dec_b"]
    return wts.astype(BF16), wtf, bias


def _install_ntff_hook():
    """The trimmed agent image lacks antenv.axon_hooks; recreate it and wire
    the ctypes NTFF profiling hook from the boot shim so trace=True works."""
    try:
        import antenv.axon_hooks  # noqa: F401
        return
    except ImportError:
        pass
    try:
        import sys
        import types
        import antenv
        mod = types.ModuleType("antenv.axon_hooks")
        state = {"h": None}
        mod.set_axon_ntff_profile_hook = lambda h: state.__setitem__("h", h)
        mod.get_axon_ntff_profile_hook = lambda: state["h"]
        sys.modules["antenv.axon_hooks"] = mod
        antenv.axon_hooks = mod
        from trn_agent_boot.trn_boot import _ntff_profile_via_ctypes
        hook = _ntff_profile_via_ctypes("/opt/axon/libaxon_pjrt.so")
        if hook is not None:
            mod.set_axon_ntff_profile_hook(hook)
    except Exception as e:  # profiling is best-effort; never break the run
        print(f"ntff hook install failed: {e}")


def kernel(**inputs):
    from concourse.bass_utils import run_bass_kernel_spmd

    obs = np.asarray(inputs["obs"], dtype=np.float32)
    hid = np.asarray(inputs["hidden_state"], dtype=np.float32)

    obs_p = np.zeros((B_PAD, IN_DIM), np.float32)
    obs_p[:B_FULL] = obs
    h_p = np.zeros((B_PAD, RNN), np.float32)
    h_p[:B_FULL] = hid
    obs_bf = obs_p.astype(BF16)
    h_bf = h_p.astype(BF16)

    wts, wtf, bias = _pack_params(inputs)

    in_maps = []
    for c in range(N_CORES):
        sl = slice(c * B_CORE, (c + 1) * B_CORE)
        in_maps.append({
            "obsT": np.ascontiguousarray(obs_bf[sl].T),
            "hT": np.ascontiguousarray(h_bf[sl].T),
            "wts": wts,
            "wtf": wtf,
            "bias": bias,
        })

    nc = _build_bass()
    import os
    trace = os.environ.get("BASS_KERNEL_TRACE", "0") == "1"
    if trace:
        _install_ntff_hook()
    br = run_bass_kernel_spmd(nc, in_maps, list(range(N_CORES)), trace=trace,
                              tmpdir=os.environ.get("BASS_KERNEL_TRACE_DIR"))
    _cached["last_results"] = br

    out = np.empty((B_PAD, N_ACT), np.float32)
    h_out = np.empty((B_PAD, RNN), np.float32)
    for c in range(N_CORES):
        sl = slice(c * B_CORE, (c + 1) * B_CORE)
        out[sl] = br.results[c]["outT"].T
        h_out[sl] = br.results[c]["h_outT"].T
    return out[:B_FULL], h_out[:B_FULL]


# revision 53
# speedup vs baseline: 1.1016x; 1.1016x over previous
"""AttentionCritic forward kernel for 8 Trainium2 NeuronCores.

Math (live part of the reference; the bi-GRU "hard attention" branch and the
1-element softmax are dead code):
    x     = relu(obs @ enc_w.T + enc_b)
    h_out = GRUCell(x, h)
    v     = relu(h_out @ v_w.T + v_b)
    out   = [h_out, v] @ dec_w.T + dec_b
    return out, h_out

Strategy: batch (65535 rows, padded to 65536) is split over 8 cores (8192
rows each). All compute is done feature-major (features on SBUF partitions,
batch streaming as the matmul free dim), so no transposes are needed on the
device: the host hands the kernel transposed bf16 inputs and re-transposes
the fp32 outputs.
"""

import numpy as np
import ml_dtypes

BF16 = ml_dtypes.bfloat16

N_CORES = 8
B_FULL = 65535
B_PAD = 65536
B_CORE = B_PAD // N_CORES  # 8192
COLS = 1024                # batch columns processed per loop iteration
N_ITERS = B_CORE // COLS

IN_DIM = 256
RNN = 128
ATT = 64
N_ACT = 10

# packed bf16 weight tensor column layout (all [K, M] matmul lhsT blocks)
W_EW0 = 0      # enc_w.T rows 0:128    [128, 128]
W_EW1 = 128    # enc_w.T rows 128:256  [128, 128]
W_XR = 256     # gru_w_ih.T cols 0:128
W_XZ = 384
W_XN = 512
W_HR = 640     # gru_w_hh.T cols 0:128
W_HZ = 768
W_HN = 896
W_TOT = 1024

# packed fp32 weight tensor (v/dec matmuls run fp32 for output precision).
# v_w.T and dec_w.T[0:128] share the rhs (h_out^T), so they are fused into one
# [128, 74] stationary block: out rows 0:64 = v preact, rows 64:74 = dec part.
F_VO = 0       # [v_w.T | dec_w.T rows 0:128] -> [128, 74]
F_DL = 74      # dec_w.T rows 128:192 -> [64, 10] (on partitions 0:64)
F_TOT = 84

# bias tensor [128, 8] fp32 column layout
B_ENC = 0      # enc_b
B_R = 1        # gru_b_ih[0:128] + gru_b_hh[0:128]
B_Z = 2
B_IHN = 3      # gru_b_ih[256:384]
B_HHN = 4      # gru_b_hh[256:384]
B_V = 5        # v_b on rows 0:64
B_DEC = 6      # dec_b on rows 0:10

_cached = {}


def _build_bass():
    if "nc" in _cached:
        return _cached["nc"]
    import concourse.bass as bass
    import concourse.bacc as bacc
    import concourse.tile as tile
    from concourse import mybir

    f32 = mybir.dt.float32
    bf16 = mybir.dt.bfloat16
    ALU = mybir.AluOpType
    ACTF = mybir.ActivationFunctionType

    nc = bacc.Bacc("TRN2", target_bir_lowering=False, debug=False)
    obsT = nc.declare_dram_parameter("obsT", [IN_DIM, B_CORE], bf16, isOutput=False)
    hT = nc.declare_dram_parameter("hT", [RNN, B_CORE], bf16, isOutput=False)
    wts_d = nc.declare_dram_parameter("wts", [128, W_TOT], bf16, isOutput=False)
    wtf_d = nc.declare_dram_parameter("wtf", [128, F_TOT], bf16, isOutput=False)
    bias_d = nc.declare_dram_parameter("bias", [128, 8], f32, isOutput=False)
    h_outT = nc.declare_dram_parameter("h_outT", [RNN, B_CORE], f32, isOutput=True)
    outT = nc.declare_dram_parameter("outT", [N_ACT, B_CORE], f32, isOutput=True)

    LOAD_COLS = 2048  # input DMA granularity (2 compute iterations)

    with tile.TileContext(nc) as tc:
        with (
            tc.tile_pool(name="const", bufs=1) as const,
            tc.tile_pool(name="io", bufs=2) as io,
            tc.tile_pool(name="work", bufs=3) as work,
            tc.tile_pool(name="psum", bufs=4, space="PSUM") as psum,
        ):
            wt = const.tile([128, W_TOT], bf16)
            wf = const.tile([128, F_TOT], bf16)
            bt = const.tile([128, 8], f32)
            nc.sync.dma_start(out=wt, in_=wts_d[:, :])
            nc.sync.dma_start(out=wf, in_=wtf_d[:, :])
            nc.sync.dma_start(out=bt, in_=bias_d[:, :])

            halves = [slice(0, 512), slice(512, 1024)]

            # PE warm-up: dummy matmuls on the weight tile while the first
            # input DMAs are in flight. Keeps the HAM clock gate at 8/8 so the
            # first real iterations run at 2.4 GHz instead of 1.2.
            ps_w = psum.tile([128, 512], f32, tag="ps", name="ps_w")
            for _ in range(12):
                nc.tensor.matmul(ps_w, wt[:, 0:128], wt[:, 0:512],
                                 start=True, stop=True)
            warm_sink = work.tile([128, 512], f32, name="warm_sink")
            nc.scalar.activation(warm_sink, ps_w, ACTF.Relu)

            def stage_b_mm(hout):
                # previous iteration's fused v+dec matmul; emitted while this
                # iteration's xt is in flight so the PE never waits on hout.
                # fused: rows 0:64 = v preact, rows 64:74 = dec_w[:, :128] part
                ps_vo = psum.tile([74, COLS], f32, tag="ps", name="ps_vo")
                for s in halves:
                    nc.tensor.matmul(ps_vo[:, s], wf[:, F_VO:F_VO + 74],
                                     hout[:, s], start=True, stop=True)
                vt = work.tile([64, COLS], bf16, name="vt")
                nc.scalar.activation(vt, ps_vo[0:64, :], ACTF.Relu,
                                     bias=bt[0:64, B_V:B_V + 1])
                return ps_vo, vt

            def stage_b_fin(ps_vo, vt, csl):
                # accumulates onto the closed group above: has_written persists
                # on HW until the next start=True; skip the sim-only check
                for s in halves:
                    nc.tensor.matmul(ps_vo[64:74, s], wf[0:64, F_DL:F_DL + 10],
                                     vt[:, s], start=False, stop=True,
                                     skip_group_check=True)
                ot = work.tile([10, COLS], f32, name="ot")
                nc.vector.tensor_scalar_add(ot, ps_vo[64:74, :],
                                            bt[0:10, B_DEC:B_DEC + 1])
                nc.sync.dma_start(out=outT[:, csl], in_=ot)

            obs0 = obs1 = htl = None
            pending = None
            pending2 = None
            for it in range(N_ITERS):
                c0 = it * COLS
                csl = slice(c0, c0 + COLS)

                if it % (LOAD_COLS // COLS) == 0:
                    lsl = slice(c0, c0 + LOAD_COLS)
                    obs0 = io.tile([128, LOAD_COLS], bf16, tag="obs0")
                    obs1 = io.tile([128, LOAD_COLS], bf16, tag="obs1")
                    htl = io.tile([128, LOAD_COLS], bf16, tag="htl")
                    nc.sync.dma_start(out=obs0, in_=obsT[0:128, lsl])
                    nc.sync.dma_start(out=obs1, in_=obsT[128:256, lsl])
                    nc.sync.dma_start(out=htl, in_=hT[:, lsl])
                w0 = (it % (LOAD_COLS // COLS)) * COLS
                o0 = obs0[:, w0:w0 + COLS]
                o1 = obs1[:, w0:w0 + COLS]
                ht = htl[:, w0:w0 + COLS]

                # x^T = relu(enc_w @ obs^T + enc_b), bf16
                # (matmuls emitted weight-major so LDWEIGHTS can be reused)
                ps_x = psum.tile([128, COLS], f32, tag="ps", name="ps_x")
                for s in halves:
                    nc.tensor.matmul(ps_x[:, s], wt[:, W_EW0:W_EW0 + 128],
                                     o0[:, s], start=True, stop=False)
                for s in halves:
                    nc.tensor.matmul(ps_x[:, s], wt[:, W_EW1:W_EW1 + 128],
                                     o1[:, s], start=False, stop=True)
                xt = work.tile([128, COLS], bf16, name="xt")
                nc.vector.tensor_scalar(out=xt, in0=ps_x,
                                        scalar1=bt[:, B_ENC:B_ENC + 1], scalar2=0.0,
                                        op0=ALU.add, op1=ALU.max)

                # GRU r/z pre-activations: h-side first (doesn't need xt)
                ps_r = psum.tile([128, COLS], f32, tag="ps", name="ps_r")
                ps_z = psum.tile([128, COLS], f32, tag="ps", name="ps_z")
                for s in halves:
                    nc.tensor.matmul(ps_r[:, s], wt[:, W_HR:W_HR + 128], ht[:, s],
                                     start=True, stop=False)
                for s in halves:
                    nc.tensor.matmul(ps_z[:, s], wt[:, W_HZ:W_HZ + 128], ht[:, s],
                                     start=True, stop=False)
                for s in halves:
                    nc.tensor.matmul(ps_r[:, s], wt[:, W_XR:W_XR + 128], xt[:, s],
                                     start=False, stop=True)
                for s in halves:
                    nc.tensor.matmul(ps_z[:, s], wt[:, W_XZ:W_XZ + 128], xt[:, s],
                                     start=False, stop=True)
                r = work.tile([128, COLS], bf16, name="r")
                nc.scalar.activation(r, ps_r, ACTF.Sigmoid, bias=bt[:, B_R:B_R + 1])
                z = work.tile([128, COLS], f32, name="z")
                nc.scalar.activation(z, ps_z, ACTF.Sigmoid, bias=bt[:, B_Z:B_Z + 1])
                # off-critical-path pieces of h_out = (1-z)*n + z*h
                z1 = work.tile([128, COLS], bf16, name="z1")
                nc.gpsimd.tensor_scalar(out=z1, in0=z, scalar1=-1.0, scalar2=1.0,
                                        op0=ALU.mult, op1=ALU.add)
                zh = work.tile([128, COLS], f32, name="zh")
                nc.gpsimd.tensor_mul(zh, z, ht)

                ps_in = psum.tile([128, COLS], f32, tag="ps", name="ps_in")
                ps_hn = psum.tile([128, COLS], f32, tag="ps", name="ps_hn")
                for s in halves:
                    nc.tensor.matmul(ps_hn[:, s], wt[:, W_HN:W_HN + 128], ht[:, s],
                                     start=True, stop=True)
                for s in halves:
                    nc.tensor.matmul(ps_in[:, s], wt[:, W_XN:W_XN + 128], xt[:, s],
                                     start=True, stop=True)

                # decoder deferred TWO iterations: hob(i-2) is guaranteed
                # complete, so the PE never waits on the DVE tail chain
                if pending2 is not None:
                    bmm = stage_b_mm(pending2[0])
                    stage_b_fin(*bmm, pending2[1])

                # n = tanh(i_n + b_ihn + r * (h_n + b_hhn))
                t = work.tile([128, COLS], f32, name="t")
                nc.vector.scalar_tensor_tensor(out=t, in0=ps_hn,
                                               scalar=bt[:, B_HHN:B_HHN + 1], in1=r,
                                               op0=ALU.add, op1=ALU.mult)
                t2 = work.tile([128, COLS], f32, name="t2")
                nc.vector.tensor_add(t2, t, ps_in)
                n = work.tile([128, COLS], bf16, name="n")
                nc.scalar.activation(n, t2, ACTF.Tanh, bias=bt[:, B_IHN:B_IHN + 1])

                # h_out = (1-z)*n + z*h
                zn = work.tile([128, COLS], bf16, name="zn")
                nc.vector.tensor_mul(zn, z1, n)
                hout = work.tile([128, COLS], f32, name="hout")
                nc.gpsimd.tensor_add(hout, zn, zh)
                nc.sync.dma_start(out=h_outT[:, csl], in_=hout)
                # bf16 twin for the decoder matmul rhs; ACT cast is cheap
                # and hob is only consumed two iterations later
                hob = work.tile([128, COLS], bf16, name="hob")
                nc.scalar.activation(hob, hout, ACTF.Copy)

                pending2 = pending
                pending = (hob, csl)

            for p in (pending2, pending):
                bmm = stage_b_mm(p[0])
                stage_b_fin(*bmm, p[1])

    nc.compile()  # bacc passes: split multi-waits into event semaphores etc.
    _cached["nc"] = nc
    return nc


def _pack_params(inputs):
    f32 = np.float32
    wts = np.zeros((128, W_TOT), f32)
    enc_wT = inputs["enc_w"].T.astype(f32)            # [256, 128]
    wts[:, W_EW0:W_EW0 + 128] = enc_wT[0:128]
    wts[:, W_EW1:W_EW1 + 128] = enc_wT[128:256]
    w_ihT = inputs["gru_w_ih"].T.astype(f32)          # [128, 384]
    w_hhT = inputs["gru_w_hh"].T.astype(f32)
    wts[:, W_XR:W_XR + 128] = w_ihT[:, 0:128]
    wts[:, W_XZ:W_XZ + 128] = w_ihT[:, 128:256]
    wts[:, W_XN:W_XN + 128] = w_ihT[:, 256:384]
    wts[:, W_HR:W_HR + 128] = w_hhT[:, 0:128]
    wts[:, W_HZ:W_HZ + 128] = w_hhT[:, 128:256]
    wts[:, W_HN:W_HN + 128] = w_hhT[:, 256:384]

    wtf = np.zeros((128, F_TOT), f32)
    wtf[:, F_VO:F_VO + 64] = inputs["v_w"].T.astype(f32)
    dec_wT = inputs["dec_w"].T.astype(f32)            # [192, 10]
    wtf[:, F_VO + 64:F_VO + 74] = dec_wT[0:128]
    wtf[0:64, F_DL:F_DL + 10] = dec_wT[128:192]
    wtf = wtf.astype(BF16)

    bias = np.zeros((128, 8), f32)
    bias[:, B_ENC] = inputs["enc_b"]
    b_ih = inputs["gru_b_ih"].astype(f32)
    b_hh = inputs["gru_b_hh"].astype(f32)
    bias[:, B_R] = b_ih[0:128] + b_hh[0:128]
    bias[:, B_Z] = b_ih[128:256] + b_hh[128:256]
    bias[:, B_IHN] = b_ih[256:384]
    bias[:, B_HHN] = b_hh[256:384]
    bias[0:64, B_V] = inputs["v_b"]
    bias[0:10, B_DEC] = inputs["dec_b"]
    return wts.astype(BF16), wtf, bias


def _install_ntff_hook():
    """The trimmed agent image lacks antenv.axon_hooks; recreate it and wire
    the ctypes NTFF profiling hook from the boot shim so trace=True works."""
    try:
        import antenv.axon_hooks  # noqa: F401
        return
    except ImportError:
        pass
    try:
        import sys
        import types
        import antenv
        mod = types.ModuleType("antenv.axon_hooks")
        state = {"h": None}
        mod.set_axon_ntff_profile_hook = lambda h: state.__setitem__("h", h)
        mod.get_axon_ntff_profile_hook = lambda: state["h"]
        sys.modules["antenv.axon_hooks"] = mod
        antenv.axon_hooks = mod
        from trn_agent_boot.trn_boot import _ntff_profile_via_ctypes
        hook = _ntff_profile_via_ctypes("/opt/axon/libaxon_pjrt.so")
        if hook is not None:
            mod.set_axon_ntff_profile_hook(hook)
    except Exception as e:  # profiling is best-effort; never break the run
        print(f"ntff hook install failed: {e}")


def kernel(**inputs):
    from concourse.bass_utils import run_bass_kernel_spmd

    obs = np.asarray(inputs["obs"], dtype=np.float32)
    hid = np.asarray(inputs["hidden_state"], dtype=np.float32)

    obs_p = np.zeros((B_PAD, IN_DIM), np.float32)
    obs_p[:B_FULL] = obs
    h_p = np.zeros((B_PAD, RNN), np.float32)
    h_p[:B_FULL] = hid
    obs_bf = obs_p.astype(BF16)
    h_bf = h_p.astype(BF16)

    wts, wtf, bias = _pack_params(inputs)

    in_maps = []
    for c in range(N_CORES):
        sl = slice(c * B_CORE, (c + 1) * B_CORE)
        in_maps.append({
            "obsT": np.ascontiguousarray(obs_bf[sl].T),
            "hT": np.ascontiguousarray(h_bf[sl].T),
            "wts": wts,
            "wtf": wtf,
            "bias": bias,
        })

    nc = _build_bass()
    import os
    trace = os.environ.get("BASS_KERNEL_TRACE", "0") == "1"
    if trace:
        _install_ntff_hook()
    br = run_bass_kernel_spmd(nc, in_maps, list(range(N_CORES)), trace=trace,
                              tmpdir=os.environ.get("BASS_KERNEL_TRACE_DIR"))
    _cached["last_results"] = br

    out = np.empty((B_PAD, N_ACT), np.float32)
    h_out = np.empty((B_PAD, RNN), np.float32)
    for c in range(N_CORES):
        sl = slice(c * B_CORE, (c + 1) * B_CORE)
        out[sl] = br.results[c]["outT"].T
        h_out[sl] = br.results[c]["h_outT"].T
    return out[:B_FULL], h_out[:B_FULL]
